# revision 3
# baseline (speedup 1.0000x reference)
"""GRU-D style GRUI encoder kernel for Trainium2 (Bass/Tile), 8 NeuronCores.

Data-parallel over batch B=256 across 8 cores (32 seqs/core), with the 32
sequences split into two groups A/B of 16 that are software-pipelined at
sub-step granularity so the serial per-step dependency chain of one group
hides under engine work of the other.

Key structure vs the naive version:
  - beta = exp(-relu(delta@Wtd+b)) for ALL T steps is computed upfront
    (no ACT-table thrash between Exp and Sigmoid in the steady loop).
  - per step each group has ONE psum region [128, 6, 16] = [r0 r1 m0 m1
    h0 h1] x 16batch, seeded by a single identity-inject matmul.
  - r|mu sigmoid merged into one ACT op per group; tanh separate.
  - state update uses hb' = q + w*hhat with w = beta*mu, q = (beta-w)*hb
    computed off the critical path on GPSIMD.
  - weight tiles are contiguous [128,128] slices and each LDWEIGHTS is
    shared by the two groups' matmuls (InstMatmult.ldweights=False).
"""

import numpy as np
import ml_dtypes
from contextlib import ExitStack

import concourse.bass as bass
import concourse.bacc as bacc
import concourse.tile as tile
from concourse import mybir
from concourse.bass_utils import run_bass_kernel_spmd
from concourse.masks import make_identity

B, T, D, H = 256, 512, 128, 256
NCORES = 8
BL = B // NCORES          # 32 sequences per core
GB = 16                   # sequences per pipeline group (2 groups)
C = 64                    # steps per chunk
NCHUNK = T // C
QSTEPS = 16               # steps per precompute GEMM group (N = 16*32 = 512)

SHARE_LDW = True

FP32 = mybir.dt.float32
BF16 = mybir.dt.bfloat16
AF = mybir.ActivationFunctionType
ALU = mybir.AluOpType

_cache = {}


def _build():
    nc = bacc.Bacc("TRN2", target_bir_lowering=False, debug=False,
                   num_devices=NCORES)

    xT = nc.dram_tensor("xT", [D, T * BL], BF16, kind="ExternalInput")
    dTs = nc.dram_tensor("dTs", [D, T * BL], BF16, kind="ExternalInput")
    wx_rmu_d = nc.dram_tensor("wx_rmu", [D, 2 * H], BF16, kind="ExternalInput")
    wx_h_d = nc.dram_tensor("wx_h", [D, H], BF16, kind="ExternalInput")
    wtd_d = nc.dram_tensor("wtd", [D, H], BF16, kind="ExternalInput")
    wh_all_d = nc.dram_tensor("wh_all", [128, 12 * 128], BF16,
                              kind="ExternalInput")
    b_rmu_d = nc.dram_tensor("b_rmu", [128, 4], FP32, kind="ExternalInput")
    b_h_d = nc.dram_tensor("b_h", [128, 2], FP32, kind="ExternalInput")
    nb_td_d = nc.dram_tensor("nb_td", [128, 2], FP32, kind="ExternalInput")
    out_d = nc.dram_tensor("hT_out", [128, 2 * BL], FP32, kind="ExternalOutput")

    def mm(out, w_ap, rhs, start, stop):
        inst = nc.tensor.matmul(out, w_ap, rhs, start=start, stop=stop)
        if SHARE_LDW:
            inst.ins.ldweights = False
        return inst

    def ldw(w_ap):
        if SHARE_LDW:
            nc.tensor.ldweights(w_ap)

    with ExitStack() as ctx:
        tc = ctx.enter_context(tile.TileContext(nc))
        wpool = ctx.enter_context(tc.tile_pool(name="weights", bufs=1))
        betapool = ctx.enter_context(tc.tile_pool(name="beta", bufs=1))
        xpool = ctx.enter_context(tc.tile_pool(name="xin", bufs=2))
        gxpool = ctx.enter_context(tc.tile_pool(name="gx", bufs=2))
        pre_ps = ctx.enter_context(tc.tile_pool(name="pre_ps", bufs=2,
                                                space="PSUM"))
        sps_pool = ctx.enter_context(tc.tile_pool(name="sps", bufs=2,
                                                  space="PSUM"))
        spool = ctx.enter_context(tc.tile_pool(name="state", bufs=3))

        # --- weights / constants into SBUF ---
        wh_all = wpool.tile([128, 12, 128], BF16)
        nc.sync.dma_start(wh_all.rearrange("p a b -> p (a b)"), wh_all_d[:, :])
        wx_rmu = wpool.tile([128, 2 * H], BF16)
        nc.sync.dma_start(wx_rmu, wx_rmu_d[:, :])
        wx_h = wpool.tile([128, H], BF16)
        nc.sync.dma_start(wx_h, wx_h_d[:, :])
        wtd = wpool.tile([128, H], BF16)
        nc.sync.dma_start(wtd, wtd_d[:, :])
        b_rmu = wpool.tile([128, 4], FP32)
        nc.sync.dma_start(b_rmu, b_rmu_d[:, :])
        b_h = wpool.tile([128, 2], FP32)
        nc.sync.dma_start(b_h, b_h_d[:, :])
        nb_td = wpool.tile([128, 2], FP32)
        nc.sync.dma_start(nb_td, nb_td_d[:, :])
        ident = wpool.tile([128, 128], BF16)
        make_identity(nc, ident)

        # Touch bias tiles from DVE once so later TSP/ACT consumers don't
        # carry the DMA wait (walrus rejects TSP with 2 sync waits).
        scratch = wpool.tile([128, 4], FP32, tag="scratch")
        nc.vector.tensor_copy(scratch, b_rmu)
        scratch2 = wpool.tile([128, 2], FP32, tag="scratch2")
        nc.vector.tensor_copy(scratch2, b_h)
        scratch3 = wpool.tile([128, 2], FP32, tag="scratch3")
        nc.vector.tensor_copy(scratch3, nb_td)

        # beta for every step: [p, t, gb, k, b]
        bet_all = betapool.tile([128, T, 2, 2, GB], BF16)

        # ---------- Phase 1: all temporal-decay betas upfront ----------
        for c in range(NCHUNK):
            dch = xpool.tile([128, C * BL], BF16, tag="dch")
            nc.sync.dma_start(dch, dTs[:, c * C * BL:(c + 1) * C * BL])
            for q in range(C // QSTEPS):
                nsl = slice(q * QSTEPS * BL, (q + 1) * QSTEPS * BL)
                for k in range(2):
                    bps = pre_ps.tile([128, QSTEPS * BL], FP32, tag="bps")
                    nc.tensor.matmul(bps, wtd[:, k * 128:(k + 1) * 128],
                                     dch[:, nsl], start=True, stop=True)
                    # exp(-(z + b)) ; clamped to <=1 below (== exp(-relu))
                    nc.scalar.activation(
                        bet_all[:, c * C + q * QSTEPS:
                                c * C + (q + 1) * QSTEPS, :, k, :],
                        bps.rearrange("p (t g b) -> p t g b", g=2, b=GB),
                        AF.Exp, bias=nb_td[:, k:k + 1], scale=-1.0)
            nc.vector.tensor_scalar_min(
                bet_all[:, c * C:(c + 1) * C].rearrange(
                    "p t g k b -> p (t g k b)"),
                bet_all[:, c * C:(c + 1) * C].rearrange(
                    "p t g k b -> p (t g k b)"), 1.0)

        # ---------- Phase 2: gate-x precompute + recurrence ----------
        def emit_pre_unit(c, u):
            """One x-GEMM + evacuation for chunk c, unit u (0..23)."""
            q, m = divmod(u, 6)
            nsl = slice(q * QSTEPS * BL, (q + 1) * QSTEPS * BL)
            tsl = slice(q * QSTEPS, (q + 1) * QSTEPS)
            ps = pre_ps.tile([128, QSTEPS * BL], FP32, tag="bps")
            if m < 4:
                lhsT = wx_rmu[:, m * 128:(m + 1) * 128]
            else:
                lhsT = wx_h[:, (m - 4) * 128:(m - 4 + 1) * 128]
            nc.tensor.matmul(ps, lhsT, cur_x[:, nsl], start=True, stop=True)
            src = ps.rearrange("p (t g b) -> p t g b", g=2, b=GB)
            dest = cur_gx[:, tsl, :, m, :]
            if m < 4:
                nc.vector.tensor_scalar_add(dest, src, b_rmu[:, m:m + 1])
            else:
                nc.scalar.activation(dest, src, AF.Identity,
                                     bias=b_h[:, m - 4:m - 4 + 1])

        # precompute-unit emission schedule within the previous chunk
        sched = {}
        for u in range(24):
            sched.setdefault(2 + (u * 5) // 2, []).append(u)

        # initial state (= beta_0 * h_{-1} = 0)
        hb = spool.tile([128, 2, 2, GB], BF16, tag="hb")
        nc.vector.memset(hb.rearrange("p a k b -> p (a k b)"), 0.0)

        # chunk 0 x + precompute fully upfront
        cur_x = xpool.tile([128, C * BL], BF16, tag="xch")
        nc.sync.dma_start(cur_x, xT[:, 0:C * BL])
        cur_gx = gxpool.tile([128, C, 2, 6, GB], BF16, tag="gx")
        for u in range(24):
            emit_pre_unit(0, u)

        for c in range(NCHUNK):
            gx = cur_gx
            xch = cur_x
            if c + 1 < NCHUNK:
                cur_x = xpool.tile([128, C * BL], BF16, tag="xch")
                nc.sync.dma_start(cur_x,
                                  xT[:, (c + 1) * C * BL:(c + 2) * C * BL])
                cur_gx = gxpool.tile([128, C, 2, 6, GB], BF16, tag="gx")

            for i in range(C):
                t = c * C + i
                last = (t == T - 1)
                bet_t = bet_all[:, t]          # [p, gb, k, b]

                sps_a = sps_pool.tile([128, 6, GB], FP32, tag="sA")
                sps_b = sps_pool.tile([128, 6, GB], FP32, tag="sB")
                sps = [sps_a, sps_b]

                # inject precomputed x-parts (+bias) into PSUM
                ldw(ident[:, :])
                for g in range(2):
                    mm(sps[g], ident[:, :], gx[:, i, g], start=True,
                       stop=False)

                # r then mu gate matmuls (weight tile shared by both groups)
                for m in range(4):
                    for k in range(2):
                        j = m * 2 + k
                        w_ap = wh_all[:, j, :]
                        ldw(w_ap)
                        for g in range(2):
                            mm(sps[g][:, m, :], w_ap, hb[:, g, k, :],
                               start=False, stop=False)

                # merged r|mu sigmoid per group
                rm = spool.tile([128, 2, 4, GB], BF16, tag="rm")
                for g in range(2):
                    nc.scalar.activation(rm[:, g], sps[g][:, 0:4, :],
                                         AF.Sigmoid)

                # rh = r * hb  (on critical path, DVE)
                rh = []
                for g in range(2):
                    rh_g = spool.tile([128, 2, GB], BF16, tag=f"rh{g}")
                    nc.vector.tensor_mul(rh_g, rm[:, g, 0:2, :], hb[:, g])
                    rh.append(rh_g)

                # off-critical-path gate algebra on GPSIMD
                w_t = spool.tile([128, 2, 2, GB], BF16, tag="w")
                q_t = spool.tile([128, 2, 2, GB], BF16, tag="q")
                mu_view = rm[:, :, 2:4, :]
                if not last:
                    nc.gpsimd.tensor_mul(w_t, bet_t, mu_view)
                    t1 = spool.tile([128, 2, 2, GB], BF16, tag="t1")
                    nc.gpsimd.tensor_sub(t1, bet_t, w_t)
                    nc.gpsimd.tensor_mul(q_t, t1, hb)
                else:
                    # last step: no decay ahead; w = mu, q = (1-mu)*hb
                    nc.gpsimd.tensor_copy(w_t, mu_view)
                    t1 = spool.tile([128, 2, 2, GB], BF16, tag="t1")
                    nc.gpsimd.tensor_scalar(t1, mu_view, -1.0, 1.0,
                                            ALU.mult, ALU.add)
                    nc.gpsimd.tensor_mul(q_t, t1, hb)

                # h_hat matmuls
                for m in range(2):
                    for k in range(2):
                        j = 8 + m * 2 + k
                        w_ap = wh_all[:, j, :]
                        ldw(w_ap)
                        for g in range(2):
                            mm(sps[g][:, 4 + m, :], w_ap, rh[g][:, k, :],
                               start=False, stop=(m == 1 and k == 1))

                hhat = spool.tile([128, 2, 2, GB], BF16, tag="hh")
                for g in range(2):
                    nc.scalar.activation(hhat[:, g], sps[g][:, 4:6, :],
                                         AF.Tanh)

                # state update: hb' = q + w * hhat
                if not last:
                    hb_new = spool.tile([128, 2, 2, GB], BF16, tag="hb")
                    for g in range(2):
                        e_g = spool.tile([128, 2, GB], BF16, tag=f"e{g}")
                        nc.vector.tensor_mul(e_g, w_t[:, g], hhat[:, g])
                        nc.vector.tensor_add(hb_new[:, g], q_t[:, g], e_g)
                    hb = hb_new
                else:
                    hout = spool.tile([128, 2, 2, GB], FP32, tag="ho")
                    for g in range(2):
                        e_g = spool.tile([128, 2, GB], BF16, tag=f"e{g}")
                        nc.vector.tensor_mul(e_g, w_t[:, g], hhat[:, g])
                        nc.vector.tensor_add(hout[:, g], q_t[:, g], e_g)
                    nc.sync.dma_start(
                        out_d[:, :], hout.rearrange("p a k b -> p (a k b)"))

                # spread next chunk's precompute over this chunk's steps
                if c + 1 < NCHUNK and i in sched:
                    for u in sched[i]:
                        emit_pre_unit(c + 1, u)

    nc.compile()
    return nc


def _prep_inputs(x, delta, W_mu, b_mu, W_r, b_r, W_h, b_h, W_td, b_td):
    bf = ml_dtypes.bfloat16
    # weights: first H rows act on h, last D rows act on x
    wh_gates = [W_r[:H], W_mu[:H], W_h[:H]]          # each [256, 256]
    wx_rmu = np.concatenate([W_r[H:], W_mu[H:]], axis=1)      # [128, 512]
    wx_h = W_h[H:]

    # contiguous [128,128] weight tiles: j = gate-major (r0,r1,mu0,mu1,h0,h1)
    # with k (contraction tile) minor
    tiles = []
    for gi, m in ((0, 0), (0, 1), (1, 0), (1, 1), (2, 0), (2, 1)):
        for k in range(2):
            tiles.append(wh_gates[gi][k * 128:(k + 1) * 128,
                                      m * 128:(m + 1) * 128])
    wh_all = np.concatenate(tiles, axis=1)                    # [128, 1536]

    def pcol(v):  # [2*128] -> [128, 2] column-per-tile
        return np.ascontiguousarray(np.stack([v[:128], v[128:]], axis=1),
                                    dtype=np.float32)

    b_rmu_col = np.concatenate([b_r, b_mu])                    # [512]
    b_rmu_t = np.ascontiguousarray(
        np.stack([b_rmu_col[i * 128:(i + 1) * 128] for i in range(4)], axis=1),
        dtype=np.float32)                                      # [128, 4]

    shared = {
        "wx_rmu": np.ascontiguousarray(wx_rmu, dtype=bf),
        "wx_h": np.ascontiguousarray(wx_h, dtype=bf),
        "wtd": np.ascontiguousarray(W_td, dtype=bf),
        "wh_all": np.ascontiguousarray(wh_all, dtype=bf),
        "b_rmu": b_rmu_t,
        "b_h": pcol(b_h),
        "nb_td": pcol(-b_td),
    }

    # delta shifted by one step: beta used at step t is beta(t+1)
    dshift = np.concatenate(
        [delta[:, 1:, :], np.zeros((B, 1, D), np.float32)], axis=1)

    in_maps = []
    for ci in range(NCORES):
        xs = x[ci * BL:(ci + 1) * BL]          # [32, 512, 128]
        ds = dshift[ci * BL:(ci + 1) * BL]
        # [BL, T, D] -> [D, T, BL] -> [D, T*BL]  (column t*BL + b)
        xt = np.ascontiguousarray(
            xs.transpose(2, 1, 0).reshape(D, T * BL), dtype=bf)
        dt_ = np.ascontiguousarray(
            ds.transpose(2, 1, 0).reshape(D, T * BL), dtype=bf)
        in_maps.append({"xT": xt, "dTs": dt_, **shared})
    return in_maps


def kernel(x, delta, W_mu, b_mu, W_r, b_r, W_h, b_h, W_td, b_td):
    args = tuple(np.asarray(a, dtype=np.float32) for a in
                 (x, delta, W_mu, b_mu, W_r, b_r, W_h, b_h, W_td, b_td))
    in_maps = _prep_inputs(*args)
    if "nc" not in _cache:
        _cache["nc"] = _build()
    res = run_bass_kernel_spmd(_cache["nc"], in_maps,
                               core_ids=list(range(NCORES)))
    out = np.empty((B, H), np.float32)
    for ci in range(NCORES):
        o = res.results[ci]["hT_out"]          # [128, 2*BL]; col = g*32+k*16+b
        for g in range(2):
            for k in range(2):
                out[ci * BL + g * GB:ci * BL + (g + 1) * GB,
                    k * 128:(k + 1) * 128] = \
                    o[:, g * 32 + k * GB:g * 32 + (k + 1) * GB].T
    return out


# revision 4
# speedup vs baseline: 1.0405x; 1.0405x over previous
"""GRU-D style GRUI encoder kernel for Trainium2 (Bass/Tile), 8 NeuronCores.

Data-parallel over batch B=256 across 8 cores (32 sequences/core). State is
kept hidden-on-partitions: hb[p, k, b] with k the hidden 128-tile and b the
batch lane, so the recurrence matmuls use stationary weights and stream the
32 batch columns.

Optimizations vs the naive version:
  - beta = exp(-relu(delta@Wtd+b)) for ALL T steps is computed upfront
    (no ACT-table thrash between Exp and Sigmoid in the steady loop).
  - per step ONE psum region [128, 6, 32] = [r0 r1 m0 m1 h0 h1] seeded by a
    single identity-inject matmul (1 instead of 3).
  - r sigmoid split per hidden tile so rh and the h_hat matmuls start as
    early as possible; mu/tanh are single ops.
  - state update hb' = q + w*hhat with w = beta*mu, t1 = beta-w, q = t1*hb
    computed off the critical path on GPSIMD; hb' written k-tile-first so
    the next step's first matmuls launch before the second half lands.
  - gate-x GEMMs for chunk c+1 are spread one-per-step across chunk c.
"""

import numpy as np
import ml_dtypes
from contextlib import ExitStack

import concourse.bass as bass
import concourse.bacc as bacc
import concourse.tile as tile
from concourse import mybir
from concourse.bass_utils import run_bass_kernel_spmd
from concourse.masks import make_identity

B, T, D, H = 256, 512, 128, 256
NCORES = 8
BL = B // NCORES          # 32 sequences per core
C = 64                    # steps per chunk
NCHUNK = T // C
QSTEPS = 16               # steps per precompute GEMM group (N = 16*32 = 512)

FP32 = mybir.dt.float32
BF16 = mybir.dt.bfloat16
AF = mybir.ActivationFunctionType
ALU = mybir.AluOpType

_cache = {}


def _build():
    nc = bacc.Bacc("TRN2", target_bir_lowering=False, debug=False,
                   num_devices=NCORES)

    xT = nc.dram_tensor("xT", [D, T * BL], BF16, kind="ExternalInput")
    dTs = nc.dram_tensor("dTs", [D, T * BL], BF16, kind="ExternalInput")
    wx_rmu_d = nc.dram_tensor("wx_rmu", [D, 2 * H], BF16, kind="ExternalInput")
    wx_h_d = nc.dram_tensor("wx_h", [D, H], BF16, kind="ExternalInput")
    wtd_d = nc.dram_tensor("wtd", [D, H], BF16, kind="ExternalInput")
    wh_all_d = nc.dram_tensor("wh_all", [128, 12 * 128], BF16,
                              kind="ExternalInput")
    b_rmu_d = nc.dram_tensor("b_rmu", [128, 4], FP32, kind="ExternalInput")
    b_h_d = nc.dram_tensor("b_h", [128, 2], FP32, kind="ExternalInput")
    nb_td_d = nc.dram_tensor("nb_td", [128, 2], FP32, kind="ExternalInput")
    out_d = nc.dram_tensor("hT_out", [128, 2 * BL], FP32, kind="ExternalOutput")

    with ExitStack() as ctx:
        tc = ctx.enter_context(tile.TileContext(nc))
        wpool = ctx.enter_context(tc.tile_pool(name="weights", bufs=1))
        betapool = ctx.enter_context(tc.tile_pool(name="beta", bufs=1))
        xpool = ctx.enter_context(tc.tile_pool(name="xin", bufs=2))
        gxpool = ctx.enter_context(tc.tile_pool(name="gx", bufs=2))
        pre_ps = ctx.enter_context(tc.tile_pool(name="pre_ps", bufs=2,
                                                space="PSUM"))
        sps_pool = ctx.enter_context(tc.tile_pool(name="sps", bufs=2,
                                                  space="PSUM"))
        spool = ctx.enter_context(tc.tile_pool(name="state", bufs=3))

        # --- weights / constants into SBUF ---
        wh_all = wpool.tile([128, 12, 128], BF16)
        nc.sync.dma_start(wh_all.rearrange("p a b -> p (a b)"), wh_all_d[:, :])
        wx_rmu = wpool.tile([128, 2 * H], BF16)
        nc.sync.dma_start(wx_rmu, wx_rmu_d[:, :])
        wx_h = wpool.tile([128, H], BF16)
        nc.sync.dma_start(wx_h, wx_h_d[:, :])
        wtd = wpool.tile([128, H], BF16)
        nc.sync.dma_start(wtd, wtd_d[:, :])
        b_rmu = wpool.tile([128, 4], FP32)
        nc.sync.dma_start(b_rmu, b_rmu_d[:, :])
        b_h = wpool.tile([128, 2], FP32)
        nc.sync.dma_start(b_h, b_h_d[:, :])
        nb_td = wpool.tile([128, 2], FP32)
        nc.sync.dma_start(nb_td, nb_td_d[:, :])
        ident = wpool.tile([128, 128], BF16)
        make_identity(nc, ident)

        # Touch bias tiles from DVE once so later TSP/ACT consumers don't
        # carry the DMA wait (walrus rejects TSP with 2 sync waits).
        scratch = wpool.tile([128, 4], FP32, tag="scratch")
        nc.vector.tensor_copy(scratch, b_rmu)
        scratch2 = wpool.tile([128, 2], FP32, tag="scratch2")
        nc.vector.tensor_copy(scratch2, b_h)
        scratch3 = wpool.tile([128, 2], FP32, tag="scratch3")
        nc.vector.tensor_copy(scratch3, nb_td)

        # beta for every step: [p, t, k, b]
        bet_all = betapool.tile([128, T, 2, BL], BF16)

        # ---------- Phase 1: all temporal-decay betas upfront ----------
        for c in range(NCHUNK):
            dch = xpool.tile([128, C * BL], BF16, tag="dch")
            nc.sync.dma_start(dch, dTs[:, c * C * BL:(c + 1) * C * BL])
            for q in range(C // QSTEPS):
                nsl = slice(q * QSTEPS * BL, (q + 1) * QSTEPS * BL)
                for k in range(2):
                    bps = pre_ps.tile([128, QSTEPS * BL], FP32, tag="bps")
                    nc.tensor.matmul(bps, wtd[:, k * 128:(k + 1) * 128],
                                     dch[:, nsl], start=True, stop=True)
                    # exp(-(z + b)) ; clamped to <=1 below (== exp(-relu))
                    nc.scalar.activation(
                        bet_all[:, c * C + q * QSTEPS:
                                c * C + (q + 1) * QSTEPS, k, :],
                        bps.rearrange("p (t b) -> p t b", b=BL),
                        AF.Exp, bias=nb_td[:, k:k + 1], scale=-1.0)
            nc.vector.tensor_scalar_min(
                bet_all[:, c * C:(c + 1) * C].rearrange(
                    "p t k b -> p (t k b)"),
                bet_all[:, c * C:(c + 1) * C].rearrange(
                    "p t k b -> p (t k b)"), 1.0)

        # ---------- Phase 2: gate-x precompute + recurrence ----------
        def emit_pre_unit(c, u):
            """One x-GEMM + evacuation for chunk c, unit u (0..23)."""
            q, m = divmod(u, 6)
            nsl = slice(q * QSTEPS * BL, (q + 1) * QSTEPS * BL)
            tsl = slice(q * QSTEPS, (q + 1) * QSTEPS)
            ps = pre_ps.tile([128, QSTEPS * BL], FP32, tag="bps")
            if m < 4:
                lhsT = wx_rmu[:, m * 128:(m + 1) * 128]
            else:
                lhsT = wx_h[:, (m - 4) * 128:(m - 4 + 1) * 128]
            nc.tensor.matmul(ps, lhsT, cur_x[:, nsl], start=True, stop=True)
            src = ps.rearrange("p (t b) -> p t b", b=BL)
            dest = cur_gx[:, tsl, m, :]
            if m < 4:
                nc.vector.tensor_scalar_add(dest, src, b_rmu[:, m:m + 1])
            else:
                nc.scalar.activation(dest, src, AF.Identity,
                                     bias=b_h[:, m - 4:m - 4 + 1])

        # precompute-unit emission schedule within the previous chunk
        sched = {}
        for u in range(24):
            sched.setdefault(2 + (u * 5) // 2, []).append(u)

        # initial state (= beta_0 * h_{-1} = 0)
        hb = spool.tile([128, 2, BL], BF16, tag="hb")
        nc.vector.memset(hb.rearrange("p k b -> p (k b)"), 0.0)

        # chunk 0 x + precompute fully upfront
        cur_x = xpool.tile([128, C * BL], BF16, tag="xch")
        nc.sync.dma_start(cur_x, xT[:, 0:C * BL])
        cur_gx = gxpool.tile([128, C, 6, BL], BF16, tag="gx")
        for u in range(24):
            emit_pre_unit(0, u)

        for c in range(NCHUNK):
            gx = cur_gx
            if c + 1 < NCHUNK:
                cur_x = xpool.tile([128, C * BL], BF16, tag="xch")
                nc.sync.dma_start(cur_x,
                                  xT[:, (c + 1) * C * BL:(c + 2) * C * BL])
                cur_gx = gxpool.tile([128, C, 6, BL], BF16, tag="gx")

            for i in range(C):
                t = c * C + i
                last = (t == T - 1)
                bet_t = bet_all[:, t]          # [p, k, b]

                sps = sps_pool.tile([128, 6, BL], FP32, tag="sps")

                # inject precomputed x-parts (+bias) into PSUM
                nc.tensor.matmul(sps, ident[:, :], gx[:, i], start=True,
                                 stop=False)

                # r gate matmuls, m-major so r_m0 activates early
                for m in range(2):
                    for k in range(2):
                        nc.tensor.matmul(sps[:, m, :], wh_all[:, m * 2 + k, :],
                                         hb[:, k, :], start=False, stop=False)
                # mu gate matmuls
                for m in range(2):
                    for k in range(2):
                        nc.tensor.matmul(sps[:, 2 + m, :],
                                         wh_all[:, 4 + m * 2 + k, :],
                                         hb[:, k, :], start=False, stop=False)

                # r sigmoid per hidden tile (m == k of rh)
                r_t = spool.tile([128, 2, BL], BF16, tag="r")
                nc.scalar.activation(r_t[:, 0], sps[:, 0, :], AF.Sigmoid)
                nc.scalar.activation(r_t[:, 1], sps[:, 1, :], AF.Sigmoid)
                mu_t = spool.tile([128, 2, BL], BF16, tag="mu")
                nc.scalar.activation(mu_t, sps[:, 2:4, :], AF.Sigmoid)

                # rh = r * hb, split by k so h-matmuls start early
                rh = spool.tile([128, 2, BL], BF16, tag="rh")
                nc.vector.tensor_mul(rh[:, 0], r_t[:, 0], hb[:, 0])
                nc.vector.tensor_mul(rh[:, 1], r_t[:, 1], hb[:, 1])

                # off-critical-path gate algebra on GPSIMD
                w_t = spool.tile([128, 2, BL], BF16, tag="w")
                q_t = spool.tile([128, 2, BL], BF16, tag="q")
                t1 = spool.tile([128, 2, BL], BF16, tag="t1")
                if not last:
                    nc.gpsimd.tensor_mul(w_t, bet_t, mu_t)
                    nc.gpsimd.tensor_sub(t1, bet_t, w_t)
                    nc.gpsimd.tensor_mul(q_t, t1, hb)
                else:
                    # last step: no decay ahead; w = mu, q = (1-mu)*hb
                    nc.gpsimd.tensor_copy(w_t, mu_t)
                    nc.gpsimd.tensor_scalar(t1, mu_t, -1.0, 1.0,
                                            ALU.mult, ALU.add)
                    nc.gpsimd.tensor_mul(q_t, t1, hb)

                # h_hat matmuls, k-major so they chase rh halves
                for k in range(2):
                    for m in range(2):
                        nc.tensor.matmul(sps[:, 4 + m, :],
                                         wh_all[:, 8 + m * 2 + k, :],
                                         rh[:, k, :], start=False,
                                         stop=(m == 1 and k == 1))

                hhat = spool.tile([128, 2, BL], BF16, tag="hh")
                nc.scalar.activation(hhat, sps[:, 4:6, :], AF.Tanh)

                # state update: hb' = q + w * hhat, k0 half first
                e_t = spool.tile([128, 2, BL], BF16, tag="e")
                nc.vector.tensor_mul(e_t, w_t, hhat)
                if not last:
                    hb_new = spool.tile([128, 2, BL], BF16, tag="hb")
                    nc.vector.tensor_add(hb_new[:, 0], q_t[:, 0], e_t[:, 0])
                    nc.vector.tensor_add(hb_new[:, 1], q_t[:, 1], e_t[:, 1])
                    hb = hb_new
                else:
                    hout = spool.tile([128, 2, BL], FP32, tag="ho")
                    nc.vector.tensor_add(hout, q_t, e_t)
                    nc.sync.dma_start(
                        out_d[:, :], hout.rearrange("p k b -> p (k b)"))

                # spread next chunk's precompute over this chunk's steps
                if c + 1 < NCHUNK and i in sched:
                    for u in sched[i]:
                        emit_pre_unit(c + 1, u)

    nc.compile()
    return nc


def _prep_inputs(x, delta, W_mu, b_mu, W_r, b_r, W_h, b_h, W_td, b_td):
    bf = ml_dtypes.bfloat16
    # weights: first H rows act on h, last D rows act on x
    wh_gates = [W_r[:H], W_mu[:H], W_h[:H]]          # each [256, 256]
    wx_rmu = np.concatenate([W_r[H:], W_mu[H:]], axis=1)      # [128, 512]
    wx_h = W_h[H:]

    # contiguous [128,128] weight tiles, gate-major, k (contraction) minor
    tiles = []
    for gi, m in ((0, 0), (0, 1), (1, 0), (1, 1), (2, 0), (2, 1)):
        for k in range(2):
            tiles.append(wh_gates[gi][k * 128:(k + 1) * 128,
                                      m * 128:(m + 1) * 128])
    wh_all = np.concatenate(tiles, axis=1)                    # [128, 1536]

    def pcol(v):  # [2*128] -> [128, 2] column-per-tile
        return np.ascontiguousarray(np.stack([v[:128], v[128:]], axis=1),
                                    dtype=np.float32)

    b_rmu_col = np.concatenate([b_r, b_mu])                    # [512]
    b_rmu_t = np.ascontiguousarray(
        np.stack([b_rmu_col[i * 128:(i + 1) * 128] for i in range(4)], axis=1),
        dtype=np.float32)                                      # [128, 4]

    shared = {
        "wx_rmu": np.ascontiguousarray(wx_rmu, dtype=bf),
        "wx_h": np.ascontiguousarray(wx_h, dtype=bf),
        "wtd": np.ascontiguousarray(W_td, dtype=bf),
        "wh_all": np.ascontiguousarray(wh_all, dtype=bf),
        "b_rmu": b_rmu_t,
        "b_h": pcol(b_h),
        "nb_td": pcol(-b_td),
    }

    # delta shifted by one step: beta used at step t is beta(t+1)
    dshift = np.concatenate(
        [delta[:, 1:, :], np.zeros((B, 1, D), np.float32)], axis=1)

    in_maps = []
    for ci in range(NCORES):
        xs = x[ci * BL:(ci + 1) * BL]          # [32, 512, 128]
        ds = dshift[ci * BL:(ci + 1) * BL]
        # [BL, T, D] -> [D, T, BL] -> [D, T*BL]  (column t*BL + b)
        xt = np.ascontiguousarray(
            xs.transpose(2, 1, 0).reshape(D, T * BL), dtype=bf)
        dt_ = np.ascontiguousarray(
            ds.transpose(2, 1, 0).reshape(D, T * BL), dtype=bf)
        in_maps.append({"xT": xt, "dTs": dt_, **shared})
    return in_maps


def kernel(x, delta, W_mu, b_mu, W_r, b_r, W_h, b_h, W_td, b_td):
    args = tuple(np.asarray(a, dtype=np.float32) for a in
                 (x, delta, W_mu, b_mu, W_r, b_r, W_h, b_h, W_td, b_td))
    in_maps = _prep_inputs(*args)
    if "nc" not in _cache:
        _cache["nc"] = _build()
    res = run_bass_kernel_spmd(_cache["nc"], in_maps,
                               core_ids=list(range(NCORES)))
    out = np.empty((B, H), np.float32)
    for ci in range(NCORES):
        o = res.results[ci]["hT_out"]          # [128, 2*BL]; col = k*32 + b
        for k in range(2):
            out[ci * BL:(ci + 1) * BL, k * 128:(k + 1) * 128] = \
                o[:, k * BL:(k + 1) * BL].T
    return out


# revision 5
# speedup vs baseline: 1.1725x; 1.1269x over previous
"""GRU-D style GRUI encoder kernel for Trainium2 (Bass/Tile), 8 NeuronCores.

Data-parallel over batch B=256 across 8 cores (32 seqs/core), with the 32
sequences split into two groups A/B of 16 that are software-pipelined at
sub-step granularity: while group A's recurrence chain is in its ACT/DVE
phase, group B's matmuls run on the PE, halving the effective per-step
dependency-chain latency.

Other structure (informed by trace analysis):
  - beta = exp(-relu(delta@Wtd+b)) for ALL T steps is computed upfront
    (no ACT-table thrash between Exp and Sigmoid in the steady loop).
  - per step each group has ONE psum region [128, 6, 16] = [r0 r1 m0 m1
    h0 h1], seeded by a single identity-inject matmul.
  - r|mu sigmoid merged into one ACT op per group (ACT op count is the
    per-step budget limit); tanh separate.
  - state update hb' = p + w*(hhat-hb) with p = beta*hb computed on
    GPSIMD right at step start and w = beta*mu after mu — both off the
    critical path; the tail after tanh is 3 short DVE ops per group.
  - gate-x GEMMs for chunk c+1 are spread one-per-~2.5-steps across
    chunk c's emission so the PE never sees a burst at chunk boundaries.
"""

import numpy as np
import ml_dtypes
from contextlib import ExitStack

import concourse.bass as bass
import concourse.bacc as bacc
import concourse.tile as tile
from concourse import mybir
from concourse.bass_utils import run_bass_kernel_spmd
from concourse.masks import make_identity

B, T, D, H = 256, 512, 128, 256
NCORES = 8
BL = B // NCORES          # 32 sequences per core
GB = 16                   # sequences per pipeline group (2 groups)
C = 64                    # steps per chunk
NCHUNK = T // C
QSTEPS = 16               # steps per precompute GEMM group (N = 16*32 = 512)

FP32 = mybir.dt.float32
BF16 = mybir.dt.bfloat16
AF = mybir.ActivationFunctionType
ALU = mybir.AluOpType

_cache = {}


def _build():
    nc = bacc.Bacc("TRN2", target_bir_lowering=False, debug=False,
                   num_devices=NCORES)

    xT = nc.dram_tensor("xT", [D, T * BL], BF16, kind="ExternalInput")
    dTs = nc.dram_tensor("dTs", [D, T * BL], BF16, kind="ExternalInput")
    wx_rmu_d = nc.dram_tensor("wx_rmu", [D, 2 * H], BF16, kind="ExternalInput")
    wx_h_d = nc.dram_tensor("wx_h", [D, H], BF16, kind="ExternalInput")
    wtd_d = nc.dram_tensor("wtd", [D, H], BF16, kind="ExternalInput")
    wh_all_d = nc.dram_tensor("wh_all", [128, 12 * 128], BF16,
                              kind="ExternalInput")
    b_rmu_d = nc.dram_tensor("b_rmu", [128, 4], FP32, kind="ExternalInput")
    b_h_d = nc.dram_tensor("b_h", [128, 2], FP32, kind="ExternalInput")
    nb_td_d = nc.dram_tensor("nb_td", [128, 2], FP32, kind="ExternalInput")
    out_d = nc.dram_tensor("hT_out", [128, 2 * BL], FP32, kind="ExternalOutput")

    with ExitStack() as ctx:
        tc = ctx.enter_context(tile.TileContext(nc))
        wpool = ctx.enter_context(tc.tile_pool(name="weights", bufs=1))
        betapool = ctx.enter_context(tc.tile_pool(name="beta", bufs=1))
        xpool = ctx.enter_context(tc.tile_pool(name="xin", bufs=2))
        gxpool = ctx.enter_context(tc.tile_pool(name="gx", bufs=2))
        pre_ps = ctx.enter_context(tc.tile_pool(name="pre_ps", bufs=2,
                                                space="PSUM"))
        sps_pool = ctx.enter_context(tc.tile_pool(name="sps", bufs=2,
                                                  space="PSUM"))
        spool = ctx.enter_context(tc.tile_pool(name="state", bufs=3))

        # --- weights / constants into SBUF ---
        wh_all = wpool.tile([128, 12, 128], BF16)
        nc.sync.dma_start(wh_all.rearrange("p a b -> p (a b)"), wh_all_d[:, :])
        wx_rmu = wpool.tile([128, 2 * H], BF16)
        nc.sync.dma_start(wx_rmu, wx_rmu_d[:, :])
        wx_h = wpool.tile([128, H], BF16)
        nc.sync.dma_start(wx_h, wx_h_d[:, :])
        wtd = wpool.tile([128, H], BF16)
        nc.sync.dma_start(wtd, wtd_d[:, :])
        b_rmu = wpool.tile([128, 4], FP32)
        nc.sync.dma_start(b_rmu, b_rmu_d[:, :])
        b_h = wpool.tile([128, 2], FP32)
        nc.sync.dma_start(b_h, b_h_d[:, :])
        nb_td = wpool.tile([128, 2], FP32)
        nc.sync.dma_start(nb_td, nb_td_d[:, :])
        ident = wpool.tile([128, 128], BF16)
        make_identity(nc, ident)

        # Touch bias tiles from DVE once so later TSP/ACT consumers don't
        # carry the DMA wait (walrus rejects TSP with 2 sync waits).
        scratch = wpool.tile([128, 4], FP32, tag="scratch")
        nc.vector.tensor_copy(scratch, b_rmu)
        scratch2 = wpool.tile([128, 2], FP32, tag="scratch2")
        nc.vector.tensor_copy(scratch2, b_h)
        scratch3 = wpool.tile([128, 2], FP32, tag="scratch3")
        nc.vector.tensor_copy(scratch3, nb_td)

        # beta for every step: [p, t, gb, k, b]
        bet_all = betapool.tile([128, T, 2, 2, GB], BF16)

        # ---------- Phase 1: all temporal-decay betas upfront ----------
        for c in range(NCHUNK):
            dch = xpool.tile([128, C * BL], BF16, tag="dch")
            nc.sync.dma_start(dch, dTs[:, c * C * BL:(c + 1) * C * BL])
            for q in range(C // QSTEPS):
                nsl = slice(q * QSTEPS * BL, (q + 1) * QSTEPS * BL)
                for k in range(2):
                    bps = pre_ps.tile([128, QSTEPS * BL], FP32, tag="bps")
                    nc.tensor.matmul(bps, wtd[:, k * 128:(k + 1) * 128],
                                     dch[:, nsl], start=True, stop=True)
                    # exp(-(z + b)) ; clamped to <=1 below (== exp(-relu))
                    nc.scalar.activation(
                        bet_all[:, c * C + q * QSTEPS:
                                c * C + (q + 1) * QSTEPS, :, k, :],
                        bps.rearrange("p (t g b) -> p t g b", g=2, b=GB),
                        AF.Exp, bias=nb_td[:, k:k + 1], scale=-1.0)
            nc.vector.tensor_scalar_min(
                bet_all[:, c * C:(c + 1) * C].rearrange(
                    "p t g k b -> p (t g k b)"),
                bet_all[:, c * C:(c + 1) * C].rearrange(
                    "p t g k b -> p (t g k b)"), 1.0)

        # ---------- Phase 2: gate-x precompute + recurrence ----------
        def emit_pre_unit(c, u):
            """One x-GEMM + evacuation for chunk c, unit u (0..23)."""
            q, m = divmod(u, 6)
            nsl = slice(q * QSTEPS * BL, (q + 1) * QSTEPS * BL)
            tsl = slice(q * QSTEPS, (q + 1) * QSTEPS)
            ps = pre_ps.tile([128, QSTEPS * BL], FP32, tag="bps")
            if m < 4:
                lhsT = wx_rmu[:, m * 128:(m + 1) * 128]
            else:
                lhsT = wx_h[:, (m - 4) * 128:(m - 4 + 1) * 128]
            nc.tensor.matmul(ps, lhsT, cur_x[:, nsl], start=True, stop=True)
            src = ps.rearrange("p (t g b) -> p t g b", g=2, b=GB)
            dest = cur_gx[:, tsl, :, m, :]
            if m < 4:
                nc.vector.tensor_scalar_add(dest, src, b_rmu[:, m:m + 1])
            else:
                nc.scalar.activation(dest, src, AF.Identity,
                                     bias=b_h[:, m - 4:m - 4 + 1])

        # precompute-unit emission schedule within the previous chunk
        sched = {}
        for u in range(24):
            sched.setdefault(2 + (u * 5) // 2, []).append(u)

        # initial state (= beta_0 * h_{-1} = 0)
        hb = spool.tile([128, 2, 2, GB], BF16, tag="hb")
        nc.vector.memset(hb.rearrange("p a k b -> p (a k b)"), 0.0)

        # chunk 0 x + precompute fully upfront
        cur_x = xpool.tile([128, C * BL], BF16, tag="xch")
        nc.sync.dma_start(cur_x, xT[:, 0:C * BL])
        cur_gx = gxpool.tile([128, C, 2, 6, GB], BF16, tag="gx")
        for u in range(24):
            emit_pre_unit(0, u)

        for c in range(NCHUNK):
            gx = cur_gx
            if c + 1 < NCHUNK:
                cur_x = xpool.tile([128, C * BL], BF16, tag="xch")
                nc.sync.dma_start(cur_x,
                                  xT[:, (c + 1) * C * BL:(c + 2) * C * BL])
                cur_gx = gxpool.tile([128, C, 2, 6, GB], BF16, tag="gx")

            for i in range(C):
                t = c * C + i
                last = (t == T - 1)
                bet_t = bet_all[:, t]          # [p, gb, k, b]

                sps_a = sps_pool.tile([128, 6, GB], FP32, tag="sA")
                sps_b = sps_pool.tile([128, 6, GB], FP32, tag="sB")
                sps = (sps_a, sps_b)

                # p = beta * hb on GPSIMD, available long before the tail
                p_t = spool.tile([128, 2, 2, GB], BF16, tag="p")
                if not last:
                    nc.gpsimd.tensor_mul(p_t, bet_t, hb)

                # inject precomputed x-parts (+bias) into PSUM
                for g in range(2):
                    nc.tensor.matmul(sps[g], ident[:, :], gx[:, i, g],
                                     start=True, stop=False)

                # r then mu gate matmuls (A first, then B at each tile so
                # both groups' psums fill nearly together)
                for m in range(4):
                    for k in range(2):
                        for g in range(2):
                            nc.tensor.matmul(sps[g][:, m, :],
                                             wh_all[:, m * 2 + k, :],
                                             hb[:, g, k, :],
                                             start=False, stop=False)

                # merged r|mu sigmoid per group
                rm = spool.tile([128, 2, 4, GB], BF16, tag="rm")
                for g in range(2):
                    nc.scalar.activation(rm[:, g], sps[g][:, 0:4, :],
                                         AF.Sigmoid)

                # rh = r * hb  (on critical path, DVE)
                rh_a = spool.tile([128, 2, GB], BF16, tag="rha")
                nc.vector.tensor_mul(rh_a, rm[:, 0, 0:2, :], hb[:, 0])
                rh_b = spool.tile([128, 2, GB], BF16, tag="rhb")
                nc.vector.tensor_mul(rh_b, rm[:, 1, 0:2, :], hb[:, 1])
                rh = (rh_a, rh_b)

                # w = beta * mu on GPSIMD (ready before the tail needs it)
                w_t = spool.tile([128, 2, 2, GB], BF16, tag="w")
                mu_view = rm[:, :, 2:4, :]
                if not last:
                    nc.gpsimd.tensor_mul(w_t, bet_t, mu_view)
                else:
                    nc.gpsimd.tensor_copy(w_t, mu_view)

                # h_hat matmuls
                for k in range(2):
                    for m in range(2):
                        for g in range(2):
                            nc.tensor.matmul(sps[g][:, 4 + m, :],
                                             wh_all[:, 8 + m * 2 + k, :],
                                             rh[g][:, k, :], start=False,
                                             stop=(m == 1 and k == 1))

                hhat = spool.tile([128, 2, 2, GB], BF16, tag="hh")
                for g in range(2):
                    nc.scalar.activation(hhat[:, g], sps[g][:, 4:6, :],
                                         AF.Tanh)

                # tail: hb' = p + w*(hhat - hb), per group on DVE
                if not last:
                    hb_new = spool.tile([128, 2, 2, GB], BF16, tag="hb")
                    for g in range(2):
                        d_g = spool.tile([128, 2, GB], BF16, tag=f"d{g}")
                        nc.vector.tensor_sub(d_g, hhat[:, g], hb[:, g])
                        e_g = spool.tile([128, 2, GB], BF16, tag=f"e{g}")
                        nc.vector.tensor_mul(e_g, w_t[:, g], d_g)
                        nc.vector.tensor_add(hb_new[:, g], p_t[:, g], e_g)
                    hb = hb_new
                else:
                    # h_out = hb + mu*(hhat - hb)
                    hout = spool.tile([128, 2, 2, GB], FP32, tag="ho")
                    for g in range(2):
                        d_g = spool.tile([128, 2, GB], BF16, tag=f"d{g}")
                        nc.vector.tensor_sub(d_g, hhat[:, g], hb[:, g])
                        e_g = spool.tile([128, 2, GB], BF16, tag=f"e{g}")
                        nc.vector.tensor_mul(e_g, w_t[:, g], d_g)
                        nc.vector.tensor_add(hout[:, g], hb[:, g], e_g)
                    nc.sync.dma_start(
                        out_d[:, :], hout.rearrange("p a k b -> p (a k b)"))

                # spread next chunk's precompute over this chunk's steps
                if c + 1 < NCHUNK and i in sched:
                    for u in sched[i]:
                        emit_pre_unit(c + 1, u)

    nc.compile()
    return nc


def _prep_inputs(x, delta, W_mu, b_mu, W_r, b_r, W_h, b_h, W_td, b_td):
    bf = ml_dtypes.bfloat16
    # weights: first H rows act on h, last D rows act on x
    wh_gates = [W_r[:H], W_mu[:H], W_h[:H]]          # each [256, 256]
    wx_rmu = np.concatenate([W_r[H:], W_mu[H:]], axis=1)      # [128, 512]
    wx_h = W_h[H:]

    # contiguous [128,128] weight tiles, gate-major, k (contraction) minor
    tiles = []
    for gi, m in ((0, 0), (0, 1), (1, 0), (1, 1), (2, 0), (2, 1)):
        for k in range(2):
            tiles.append(wh_gates[gi][k * 128:(k + 1) * 128,
                                      m * 128:(m + 1) * 128])
    wh_all = np.concatenate(tiles, axis=1)                    # [128, 1536]

    def pcol(v):  # [2*128] -> [128, 2] column-per-tile
        return np.ascontiguousarray(np.stack([v[:128], v[128:]], axis=1),
                                    dtype=np.float32)

    b_rmu_col = np.concatenate([b_r, b_mu])                    # [512]
    b_rmu_t = np.ascontiguousarray(
        np.stack([b_rmu_col[i * 128:(i + 1) * 128] for i in range(4)], axis=1),
        dtype=np.float32)                                      # [128, 4]

    shared = {
        "wx_rmu": np.ascontiguousarray(wx_rmu, dtype=bf),
        "wx_h": np.ascontiguousarray(wx_h, dtype=bf),
        "wtd": np.ascontiguousarray(W_td, dtype=bf),
        "wh_all": np.ascontiguousarray(wh_all, dtype=bf),
        "b_rmu": b_rmu_t,
        "b_h": pcol(b_h),
        "nb_td": pcol(-b_td),
    }

    # delta shifted by one step: beta used at step t is beta(t+1)
    dshift = np.concatenate(
        [delta[:, 1:, :], np.zeros((B, 1, D), np.float32)], axis=1)

    in_maps = []
    for ci in range(NCORES):
        xs = x[ci * BL:(ci + 1) * BL]          # [32, 512, 128]
        ds = dshift[ci * BL:(ci + 1) * BL]
        # [BL, T, D] -> [D, T, BL] -> [D, T*BL]  (column t*BL + b)
        xt = np.ascontiguousarray(
            xs.transpose(2, 1, 0).reshape(D, T * BL), dtype=bf)
        dt_ = np.ascontiguousarray(
            ds.transpose(2, 1, 0).reshape(D, T * BL), dtype=bf)
        in_maps.append({"xT": xt, "dTs": dt_, **shared})
    return in_maps


def kernel(x, delta, W_mu, b_mu, W_r, b_r, W_h, b_h, W_td, b_td):
    args = tuple(np.asarray(a, dtype=np.float32) for a in
                 (x, delta, W_mu, b_mu, W_r, b_r, W_h, b_h, W_td, b_td))
    in_maps = _prep_inputs(*args)
    if "nc" not in _cache:
        _cache["nc"] = _build()
    res = run_bass_kernel_spmd(_cache["nc"], in_maps,
                               core_ids=list(range(NCORES)))
    out = np.empty((B, H), np.float32)
    for ci in range(NCORES):
        o = res.results[ci]["hT_out"]          # [128, 2*BL]; col = g*32+k*16+b
        for g in range(2):
            for k in range(2):
                out[ci * BL + g * GB:ci * BL + (g + 1) * GB,
                    k * 128:(k + 1) * 128] = \
                    o[:, g * 32 + k * GB:g * 32 + (k + 1) * GB].T
    return out
